# revision 2
# baseline (speedup 1.0000x reference)
"""Trainium2 Bass kernel v2: 2-layer MLP forward  y = relu(x@W1 + b1) @ W2 + b2.

Shapes: x [262144, 64], W1 [64, 128], b1 [128], W2 [128, 32], b2 [32].
Pure data parallel over 8 NeuronCores, 32768 rows per core.

Structure (per core):
  * xt shard [65, 32768] bf16 (features + a ones-row folding b1 into W1)
    fully resident in SBUF, streamed in big DMAs on the sync + gpsimd
    queues (concurrent transfer in the cost model).
  * mm1: W1b [65,128] stationary, xt moving -> h PSUM [128,1024] x3 bufs.
  * relu evac (the ACT+DVE PSUM-port bottleneck): ACT activation(Relu) /
    DVE tensor_scalar(max) per a tuned engine schedule; activation tables
    are warmed by dummy activations during the DMA fill.
  * mm2 (h_sb stationary, W2 moving) is emitted TWO groups behind mm1 so
    the in-order PE queue never blocks the next mm1 behind an mm2 that
    waits on an evacuation; y goes to PSUM banks [128,512] x2 bufs,
    evacuated by DVE tensor_tensor(+b2 pattern).
  * y ships as [128,1024] bf16 bank pairs; the final banks ship as small
    split DMAs to shorten the drain.
  * PE p-state is warmed with dummy matmuls into y bank 0 during fill.
"""

import os
import sys

import numpy as np

if "/opt/trn_rl_repo" not in sys.path:
    sys.path.insert(0, "/opt/trn_rl_repo")

N_CORES = 8
B = 262144
B_C = B // N_CORES  # 32768
N_IN, N_MID, N_OUT = 64, 128, 32

# evac groups (rows each); [128,1024] triple-buffered PSUM pool
GROUPS = [512] + [1024] * 31 + [512]
assert sum(GROUPS) == B_C
N_G = len(GROUPS)  # 33

Y_BANK_ROWS = 2048
N_YB = B_C // Y_BANK_ROWS  # 16

# x DMA segments: (queue, n_cols); sync(HWDGE) + gpsimd(SWDGE) stream
# concurrently. Boundaries are multiples of 512.
X_PLANS = {
    "c": [("sync", 512), ("sync", 1024), ("gpsimd", 2048), ("sync", 2048),
          ("gpsimd", 4096), ("sync", 8192), ("gpsimd", 8192),
          ("sync", 4096), ("gpsimd", 2560)],
    "f": [("gpsimd", 512), ("sync", 1024), ("gpsimd", 2048), ("sync", 2048),
          ("gpsimd", 4096), ("sync", 8192), ("gpsimd", 8192),
          ("sync", 4096), ("gpsimd", 2560)],
    "i": [("gpsimd", 512), ("sync", 512), ("sync", 512), ("gpsimd", 2048),
          ("sync", 2048), ("gpsimd", 4096), ("sync", 8192), ("gpsimd", 8192),
          ("sync", 4096), ("gpsimd", 2560)],
    "j": [("gpsimd", 512), ("scalar", 1024), ("gpsimd", 2048), ("sync", 2048),
          ("gpsimd", 4096), ("sync", 8192), ("gpsimd", 8192),
          ("sync", 4096), ("gpsimd", 2560)],
}
X_SEGS = X_PLANS[os.environ.get("K2_XPLAN", "j")]
assert sum(w for _, w in X_SEGS) == B_C

# engine per h evac group ("A"=ACT, "V"=DVE)
G_EVAC = os.environ.get("K2_G_EVAC")
if G_EVAC is not None:
    G_EVAC = list(G_EVAC)
else:
    G_EVAC = list("VAVAAVAAVAAVAAVAAVAAVAAVAAVAAVAAA")
assert len(G_EVAC) == N_G

# engine per y bank evacuation (plain copy, either engine)
Y_ENG = list(os.environ.get("K2_Y_ENG", "VVVVVVVVVVVVVVVV"))
assert len(Y_ENG) == N_YB

# y DMA queue per bank-pair (last pair handled specially)
Y_QUEUE = ["gpsimd", "sync", "gpsimd", "sync", "gpsimd", "sync", "gpsimd"]

N_WARM_MM = int(os.environ.get("K2_WARM", "6"))

_CACHE: dict = {}


def _build_nc():
    from contextlib import ExitStack

    import concourse.bass as bass  # noqa: F401
    import concourse.tile as tile
    from concourse import bacc, mybir

    f32 = mybir.dt.float32
    bf16 = mybir.dt.bfloat16
    add = mybir.AluOpType.add
    mx = mybir.AluOpType.max

    nc = bacc.Bacc(
        "TRN2", target_bir_lowering=False, debug=False, num_devices=N_CORES
    )
    xt_d = nc.dram_tensor("xt", [N_IN + 1, B_C], bf16, kind="ExternalInput").ap()
    w1_d = nc.dram_tensor("w1", [N_IN + 1, N_MID], bf16, kind="ExternalInput").ap()
    wb_d = nc.dram_tensor(
        "wb", [N_MID, N_OUT + 512], bf16, kind="ExternalInput"
    ).ap()
    y_d = nc.dram_tensor(
        "y", [N_YB // 2, N_MID, 1024], bf16, kind="ExternalOutput"
    ).ap()

    g_start = [0]
    for r in GROUPS:
        g_start.append(g_start[-1] + r)
    yb_start = [g * Y_BANK_ROWS for g in range(N_YB + 1)]

    with tile.TileContext(nc) as tc, ExitStack() as ctx:
        consts = ctx.enter_context(tc.tile_pool(name="consts", bufs=1))
        xp = ctx.enter_context(tc.tile_pool(name="xp", bufs=1))
        hsb_pool = ctx.enter_context(tc.tile_pool(name="hsb", bufs=3))
        ysb_pool = ctx.enter_context(tc.tile_pool(name="ysb", bufs=3))
        hps_pool = ctx.enter_context(tc.tile_pool(name="hps", bufs=3, space="PSUM"))
        yps_pool = ctx.enter_context(tc.tile_pool(name="yps", bufs=2, space="PSUM"))

        # --- warmups: junk (=1.0, doubles as the mm2-bias ones row) on the
        # idle Pool engine; dummy activations load the Relu+Copy tables ---
        junk = consts.tile([N_IN, 512], bf16, name="junk")
        nc.gpsimd.memset(junk[:], 0.0)
        adum = consts.tile([N_IN, 1], bf16, name="adum")
        nc.scalar.activation(
            adum[:], junk[:, :1], mybir.ActivationFunctionType.Relu
        )

        # --- constants: w1(+b1 row) on sync HWDGE; w2+b2 pattern on the
        # scalar queue (needed later) ---
        w1t_t = consts.tile([N_IN + 1, N_MID], bf16, name="w1t_t")
        nc.sync.dma_start(out=w1t_t[:], in_=w1_d)
        w1_t = w1t_t[:]
        # --- x segment DMAs (whole shard resident in SBUF) ---
        x_tiles = []
        col = 0
        for qi, (eng, w) in enumerate(X_SEGS):
            xt_t = xp.tile([N_IN + 1, w], bf16, name=f"xseg{qi}")
            getattr(nc, eng).dma_start(out=xt_t[:], in_=xt_d[:, col : col + w])
            x_tiles.append((col, col + w, xt_t))
            col += w

        wb_t = consts.tile([N_MID, N_OUT + 512], bf16, name="wb_t")
        nc.scalar.dma_start(out=wb_t[:], in_=wb_d)
        w2_t = wb_t[:, :N_OUT]
        b2t_p = wb_t[:, N_OUT : N_OUT + 512]  # [128, 512] b2 pattern

        # y PSUM bank tiles; bank 0 doubles as the PE-warmup target
        y_tiles = [None] * N_YB
        ysb_tiles = [None] * (N_YB // 2)
        y_tiles[0] = yps_pool.tile([N_MID, 512], f32, name="y_ps0", tag="yps")
        for _ in range(N_WARM_MM):
            nc.tensor.matmul(
                y_tiles[0][:], junk[:, :N_MID], junk[:], start=True, stop=True,
            )

        def x_slice(c0, c1):
            for s0, s1, t in x_tiles:
                if c0 >= s0 and c1 <= s1:
                    return t[:, c0 - s0 : c1 - s0]
            raise AssertionError((c0, c1))

        prev_h = [None] * N_G
        state = {"next_yb": 0}

        def do_mm2(g):
            r0 = g_start[g]
            h_sb = prev_h[g]
            for j in range(GROUPS[g] // N_MID):
                row = r0 + j * N_MID
                b = row // Y_BANK_ROWS
                if y_tiles[b] is None:
                    y_tiles[b] = yps_pool.tile(
                        [N_MID, 512], f32, name=f"y_ps{b}", tag="yps"
                    )
                off = (row % Y_BANK_ROWS) // N_MID * N_OUT
                nc.tensor.matmul(
                    y_tiles[b][:, off : off + N_OUT],
                    h_sb[:, j * N_MID : (j + 1) * N_MID],
                    w2_t,
                    start=True,
                    stop=True,
                )

        def y_evac(dst, b):
            nc.vector.tensor_tensor(dst, y_tiles[b][:], b2t_p, add)

        def flush_y(done_rows):
            # banks 0..13 pair-shipped; 14 alone; 15 split in two pieces so
            # the drain chain after the last mm2 is minimal
            while state["next_yb"] < N_YB:
                b = state["next_yb"]
                half = b % 2
                bp = b // 2
                if b < 14:
                    if yb_start[b + 1] > done_rows:
                        break
                    if half == 0:
                        ysb_tiles[bp] = ysb_pool.tile(
                            [N_MID, 1024], bf16, name=f"y_sb{bp}", tag="ysb"
                        )
                    dst = ysb_tiles[bp][:, half * 512 : (half + 1) * 512]
                    y_evac(dst, b)
                    if half == 1:
                        eng = getattr(nc, Y_QUEUE[bp])
                        eng.dma_start(out=y_d[bp], in_=ysb_tiles[bp][:])
                elif b == 14:
                    if yb_start[15] > done_rows:
                        break
                    t = ysb_pool.tile(
                        [N_MID, 512], bf16, name="y_sbl0", tag="ysbl"
                    )
                    y_evac(t[:], 14)
                    nc.scalar.dma_start(out=y_d[7, :, 0:512], in_=t[:])
                else:
                    if state.get("p1") is None:
                        if done_rows >= B_C - 512:
                            # piece 1: cols 0..384 (rows 30720..32256)
                            t1 = ysb_pool.tile(
                                [N_MID, 384], bf16, name="y_sbl1", tag="ysbl"
                            )
                            nc.vector.tensor_tensor(
                                t1[:], y_tiles[15][:, 0:384],
                                b2t_p[:, 0:384], add,
                            )
                            nc.gpsimd.dma_start(
                                out=y_d[7, :, 512:896], in_=t1[:]
                            )
                            state["p1"] = True
                        else:
                            break
                    if done_rows < B_C:
                        break
                    # piece 2: last 128 cols (rows 32256..32768)
                    t2 = ysb_pool.tile(
                        [N_MID, 128], bf16, name="y_sbl2", tag="ysbl"
                    )
                    nc.vector.tensor_tensor(
                        t2[:], y_tiles[15][:, 384:512], b2t_p[:, 384:512], add
                    )
                    nc.sync.dma_start(out=y_d[7, :, 896:1024], in_=t2[:])
                    state["next_yb"] += 1
                    break
                state["next_yb"] += 1

        for g in range(N_G + 2):
            if g < N_G:
                rows = GROUPS[g]
                c0 = g_start[g]
                h_ps = hps_pool.tile([N_MID, 1024], f32, name="h_ps", tag="h")
                for q0 in range(0, rows, 512):
                    nc.tensor.matmul(
                        h_ps[:, q0 : q0 + 512],
                        w1_t,
                        x_slice(c0 + q0, c0 + q0 + 512),
                        start=True,
                        stop=True,
                    )
                cur = hsb_pool.tile([N_MID, 1024], bf16, name="h_sb", tag="hs")
                if G_EVAC[g] == "A":
                    nc.scalar.activation(
                        cur[:, :rows], h_ps[:, :rows],
                        mybir.ActivationFunctionType.Relu,
                    )
                else:
                    nc.vector.tensor_scalar(
                        cur[:, :rows], h_ps[:, :rows], 0.0, None, mx
                    )
                prev_h[g] = cur
            if g >= 2:
                do_mm2(g - 2)
                flush_y(g_start[g - 2] + GROUPS[g - 2])

    nc.compile()
    return nc


def _get_nc():
    if "nc" not in _CACHE:
        _CACHE["nc"] = _build_nc()
    return _CACHE["nc"]


def _prep_in_maps(x, W1, b1, W2, b2):
    import ml_dtypes

    bf = ml_dtypes.bfloat16
    x = np.ascontiguousarray(x, dtype=np.float32)
    xt = np.empty((N_CORES, N_IN + 1, B_C), dtype=bf)
    xt[:, :N_IN] = x.reshape(N_CORES, B_C, N_IN).transpose(0, 2, 1).astype(bf)
    xt[:, N_IN] = bf(1.0)
    w1p = np.empty((N_IN + 1, N_MID), dtype=bf)
    w1p[:N_IN] = np.ascontiguousarray(W1, dtype=np.float32).astype(bf)
    w1p[N_IN] = np.asarray(b1, dtype=np.float32).astype(bf)
    wb = np.empty((N_MID, N_OUT + 512), dtype=bf)
    wb[:, :N_OUT] = np.ascontiguousarray(W2, dtype=np.float32).astype(bf)
    b2f = np.asarray(b2, dtype=np.float32)
    wb[:, N_OUT:] = np.tile(b2f, (N_MID, 512 // N_OUT)).astype(bf)
    return [
        {"xt": xt[i], "w1": w1p, "wb": wb}
        for i in range(N_CORES)
    ]


def _unshard(results):
    outs = []
    for i in range(N_CORES):
        yd = np.asarray(results[i]["y"], dtype=np.float32)  # [8, 128, 1024]
        # row = 4096*bp + 2048*half + 128*u + p ; col = 512*half + 32*u + o
        y = (
            yd.reshape(N_YB // 2, N_MID, 2, 16, N_OUT)
            .transpose(0, 2, 3, 1, 4)
            .reshape(B_C, N_OUT)
        )
        outs.append(y)
    return np.ascontiguousarray(np.concatenate(outs, axis=0))


def run(x, W1, b1, W2, b2, trace=False):
    from concourse.bass_utils import run_bass_kernel_spmd

    nc = _get_nc()
    in_maps = _prep_in_maps(x, W1, b1, W2, b2)
    res = run_bass_kernel_spmd(nc, in_maps, list(range(N_CORES)), trace=trace)
    return _unshard(res.results), res


def kernel(x, W1, b1, W2, b2):
    y, _ = run(x, W1, b1, W2, b2, trace=False)
    return y
